# revision 14
# baseline (speedup 1.0000x reference)
"""Trainium2 Bass kernel for CombinedRepeatCausalLinear (parallel forward).

Computes out[b,e,t] = sum_s x[b,e,s] * W[s,t] + bias[t] where
  W[s,t] = mask(t>=s) * (w0[s]*d0^(t-s) + w1[t]*d1^(t-s))
for S = 2048, x of shape (8, 1024, 2048) fp32.

W is two first-order linear recurrences along t, so instead of the dense
causal GEMM we run a blocked scan over 17 column chunks (16x126 + 32).
Each x chunk tile keeps 2 spare SBUF partitions; after a tiny transfer
matmul rebuilds the scan carries from per-chunk summaries, the carries
are DMA-scattered into those spare partitions, so ONE matmul per chunk
computes local + carry-injected output:

  out[tau, r] = sum_{sig<=tau} L_k[sig,tau] * x[r, st_k+sig]
              + d0^(tau+1) * carryA[r] + w1[t] d1^(tau+1) * carryC[r]

Per r-block of 512 rows: 17 summary matmuls (accumulated into a [34,512]
PSUM tile) + 1 transfer matmul + 17 merged local matmuls = 35 PE
streams; 70 total vs 312 for the dense version. The tensor clock is
DVFS-throttled (~1.2-1.3 GHz sustained), so stream count is what
matters.

Schedule: r-halves are pipelined (rb0 column-halves of x load first; its
summaries/carries/outputs run while rb1 loads). All matmul operands are
bf16 (fp32 PSUM accumulation); x is cast bf16 on host (halves HBM
traffic); output stored bf16, upcast on host. Constants ship as one
packed DMA. Loads/stores/scatters ride only the fast SP/Activation DMA
rings. PSUM->SBUF bias-copies alternate scalar/vector engines.
"""

import numpy as np
import ml_dtypes

import concourse.bass as bass
import concourse.mybir as mybir
import concourse.tile as tile
from concourse import bacc
from concourse.bass_utils import run_bass_kernel_spmd

F32 = mybir.dt.float32
BF16 = mybir.dt.bfloat16
BF = ml_dtypes.bfloat16

B = 8
E = 1024
S = 2048
DC = 1.0
N_CORES = 8
R = (B * E) // N_CORES      # rows per core = 1024
RB = 2                      # r-blocks of 512

CHUNKS = [(126 * k, 126) for k in range(16)] + [(2016, 32)]
NC = len(CHUNKS)            # 17
NS = 2 * NC                 # 34 summary rows (A, C per chunk)

# packed-constant free-dim offsets within cstA [128, CA_W] (bf16)
MOFF = []
_off = 0
for _st, _w in CHUNKS:
    MOFF.append(_off)
    _off += _w
SOFF = _off                 # 2048: ssum_k at SOFF + NS*k, width NS
TOFF = SOFF + NS * NC       # tmat at TOFF, width NS
CA_W = TOFF + NS

_PROGRAM = None


def _build_program():
    nc = bacc.Bacc("TRN2", target_bir_lowering=False, debug=False,
                   num_devices=N_CORES)

    xT_d = nc.declare_dram_parameter("xT", [S, R], BF16, isOutput=False)
    cA_d = nc.declare_dram_parameter("cA", [128, CA_W], BF16, isOutput=False)
    biasT_d = nc.declare_dram_parameter("biasT", [128, NC], F32,
                                        isOutput=False)
    outT_d = nc.declare_dram_parameter("outT", [S, R], BF16, isOutput=True)

    ACT = mybir.ActivationFunctionType

    with tile.TileContext(nc) as tc:
        with (
            tc.tile_pool(name="xp", bufs=1) as xp,
            tc.tile_pool(name="cst", bufs=1) as cst,
            tc.tile_pool(name="sb", bufs=1) as sbp,
            tc.tile_pool(name="osb", bufs=NC) as osb,
            tc.tile_pool(name="pe", bufs=2, space="PSUM") as pep,
            tc.tile_pool(name="po", bufs=6, space="PSUM") as pop,
        ):
            cstA = cst.tile([128, CA_W], BF16, tag="cA")
            nc.gpsimd.dma_start(cstA[:], cA_d[:])
            bias_sb = cst.tile([128, NC], F32, tag="bias")
            nc.gpsimd.dma_start(bias_sb[:], biasT_d[:])

            def ring(k):
                return nc.sync if k % 2 == 0 else nc.scalar

            # ---- x loads: rb-major on the two fast rings ----
            xs = [xp.tile([w + 2, R], BF16, tag=f"x{k}", name=f"x{k}")
                  for k, (st, w) in enumerate(CHUNKS)]
            for rb in range(RB):
                for k, (st, w) in enumerate(CHUNKS):
                    ring(k).dma_start(
                        xs[k][0:w, 512 * rb:512 * (rb + 1)],
                        xT_d[st:st + w, 512 * rb:512 * (rb + 1)])

            E_ps, c_sb = {}, {}

            def emit_summary(k, rb):
                st, w = CHUNKS[k]
                nc.tensor.matmul(
                    E_ps[rb][:],
                    cstA[0:w, SOFF + NS * k:SOFF + NS * (k + 1)],
                    xs[k][0:w, 512 * rb:512 * (rb + 1)],
                    start=(k == 0), stop=(k == NC - 1))

            def emit_transfer(rb):
                # E -> SBUF(bf16) -> transfer matmul -> carries -> SBUF
                e_sb = sbp.tile([NS, 512], BF16, tag=f"esb{rb}",
                                name=f"esb{rb}")
                nc.vector.tensor_copy(e_sb[:], E_ps[rb][:])
                c_ps = pep.tile([NS, 512], F32, tag="pe", name=f"carry{rb}")
                nc.tensor.matmul(c_ps[:], cstA[0:NS, TOFF:TOFF + NS],
                                 e_sb[:], start=True, stop=True)
                t = sbp.tile([NS, 512], BF16, tag=f"csb{rb}",
                             name=f"csb{rb}")
                nc.vector.tensor_copy(t[:], c_ps[:])
                c_sb[rb] = t

            def emit_scatter(rb):
                # carries into the 2 spare partitions of each x tile
                for k, (st, w) in enumerate(CHUNKS):
                    ring(k).dma_start(
                        xs[k][w:w + 2, 512 * rb:512 * (rb + 1)],
                        c_sb[rb][2 * k:2 * k + 2, :])

            out_sb = {}

            def emit_merged(k, rb):
                st, w = CHUNKS[k]
                ps = pop.tile([w, 512], F32, tag="po", name=f"po{k}_{rb}")
                nc.tensor.matmul(ps[:],
                                 cstA[0:w + 2, MOFF[k]:MOFF[k] + w],
                                 xs[k][0:w + 2, 512 * rb:512 * (rb + 1)],
                                 start=True, stop=True)
                if k not in out_sb:
                    out_sb[k] = osb.tile([w, R], BF16, tag="osb",
                                         name=f"o{k}")
                dst = out_sb[k][:, 512 * rb:512 * (rb + 1)]
                if k % 2 == 0:
                    nc.scalar.activation(dst, ps[:], ACT.Identity,
                                         bias=bias_sb[0:w, k:k + 1])
                else:
                    nc.vector.tensor_scalar_add(dst, ps[:],
                                                bias_sb[0:w, k:k + 1])

            # ---- phase A(rb0): summaries ----
            E_ps[0] = pep.tile([NS, 512], F32, tag="pe", name="E0")
            for k in range(NC):
                emit_summary(k, 0)
            # ---- B(rb0): carries + scatter ----
            emit_transfer(0)
            emit_scatter(0)
            # ---- A(rb1) while rb0 carries settle ----
            E_ps[1] = pep.tile([NS, 512], F32, tag="pe", name="E1")
            for k in range(NC):
                emit_summary(k, 1)
            # ---- C(rb0), with B(rb1) tucked in early ----
            for k in range(NC):
                emit_merged(k, 0)
                if k == 2:
                    emit_transfer(1)
                    emit_scatter(1)
            # ---- C(rb1) + stores ----
            for k in range(NC):
                emit_merged(k, 1)
                st, w = CHUNKS[k]
                ring(k).dma_start(outT_d[st:st + w, :], out_sb[k][:])

    nc.compile()
    return nc


def _host_prep(weight, bias, decay_value):
    w0 = weight[0].astype(np.float64)
    w1 = weight[1].astype(np.float64)
    d0 = float(np.clip(np.float32(decay_value[0, 0]), 0.9, 1.0))
    d1 = float(np.clip(np.float32(decay_value[1, 0]), 0.9, 1.0))

    cA = np.zeros((128, CA_W), dtype=np.float64)
    with np.errstate(under="ignore"):
        for k, (st, w) in enumerate(CHUNKS):
            sig = np.arange(w, dtype=np.float64)[:, None]
            tau = np.arange(w, dtype=np.float64)[None, :]
            m2 = tau >= sig
            p2 = np.where(m2, tau - sig, 0.0) / DC
            w0c = w0[st:st + w]
            w1c = w1[st:st + w]
            mo = MOFF[k]
            cA[0:w, mo:mo + w] = np.where(
                m2, w0c[:, None] * d0 ** p2 + w1c[None, :] * d1 ** p2, 0.0)
            cA[w, mo:mo + w] = d0 ** ((tau[0] + 1) / DC)
            cA[w + 1, mo:mo + w] = w1c * d1 ** ((tau[0] + 1) / DC)
            so = SOFF + NS * k
            cA[0:w, so + 2 * k] = w0c * d0 ** ((w - 1 - sig[:, 0]) / DC)
            cA[0:w, so + 2 * k + 1] = d1 ** ((w - 1 - sig[:, 0]) / DC)
            for kp, (stp, wp) in enumerate(CHUNKS[:k]):
                cA[2 * kp, TOFF + 2 * k] = d0 ** ((st - stp - wp) / DC)
                cA[2 * kp + 1, TOFF + 2 * k + 1] = \
                    d1 ** ((st - stp - wp) / DC)

    biasT = np.zeros((128, NC), dtype=np.float32)
    b32 = bias.astype(np.float32)
    for k, (st, w) in enumerate(CHUNKS):
        biasT[0:w, k] = b32[st:st + w]
    return cA.astype(BF), biasT


def _make_in_maps(x, weight, bias, decay_value):
    cA, biasT = _host_prep(weight, bias, decay_value)
    x2 = np.asarray(x, dtype=np.float32).reshape(B * E, S)
    in_maps = []
    for c in range(N_CORES):
        xT_c = np.ascontiguousarray(x2[R * c:R * (c + 1), :].T.astype(BF))
        in_maps.append({"xT": xT_c, "cA": cA, "biasT": biasT})
    return in_maps


def kernel(x, weight, bias, decay_value, index=0, recurrent=0, **_):
    global _PROGRAM
    x = np.asarray(x, dtype=np.float32)
    weight = np.asarray(weight, dtype=np.float32)
    bias = np.asarray(bias, dtype=np.float32)
    decay_value = np.asarray(decay_value, dtype=np.float32)

    if _PROGRAM is None:
        _PROGRAM = _build_program()
    nc = _PROGRAM

    in_maps = _make_in_maps(x, weight, bias, decay_value)

    res = run_bass_kernel_spmd(nc, in_maps, core_ids=list(range(N_CORES)))
    out = np.empty((B * E, S), dtype=np.float32)
    for c in range(N_CORES):
        out[R * c:R * (c + 1), :] = res.results[c]["outT"].astype(np.float32).T
    return out.reshape(B, E, S)


# revision 15
# speedup vs baseline: 1.1046x; 1.1046x over previous
"""Trainium2 Bass kernel for CombinedRepeatCausalLinear (parallel forward).

Computes out[b,e,t] = sum_s x[b,e,s] * W[s,t] + bias[t] where
  W[s,t] = mask(t>=s) * (w0[s]*d0^(t-s) + w1[t]*d1^(t-s))
for S = 2048, x of shape (8, 1024, 2048) fp32.

W is two first-order linear recurrences along t, so instead of the dense
causal GEMM we run a blocked scan over 17 column chunks (16x126 + 32).
Each x chunk tile keeps 2 spare SBUF partitions; after a tiny transfer
matmul rebuilds the scan carries from per-chunk summaries, the carries
are DMA-scattered into those spare partitions, so ONE matmul per chunk
computes local + carry-injected output:

  out[tau, r] = sum_{sig<=tau} L_k[sig,tau] * x[r, st_k+sig]
              + d0^(tau+1) * carryA[r] + w1[t] d1^(tau+1) * carryC[r]

Per r-block of 512 rows: 17 summary matmuls (accumulated into a [34,512]
PSUM tile) + 1 transfer matmul + 17 merged local matmuls = 35 PE
streams; 70 total vs 312 for the dense version. The tensor clock is
DVFS-throttled (~1.2-1.3 GHz sustained), so stream count is what
matters.

Schedule: r-halves are pipelined (rb0 column-halves of x load first; its
summaries/carries/outputs run while rb1 loads). All matmul operands are
bf16 (fp32 PSUM accumulation); x is cast bf16 on host (halves HBM
traffic); output stored bf16, upcast on host. Constants ship as one
packed DMA. Loads/stores/scatters ride only the fast SP/Activation DMA
rings. PSUM->SBUF bias-copies alternate scalar/vector engines.
"""

import numpy as np
import ml_dtypes

import concourse.bass as bass
import concourse.mybir as mybir
import concourse.tile as tile
from concourse import bacc
from concourse.bass_utils import run_bass_kernel_spmd

F32 = mybir.dt.float32
BF16 = mybir.dt.bfloat16
BF = ml_dtypes.bfloat16

B = 8
E = 1024
S = 2048
DC = 1.0
N_CORES = 8
R = (B * E) // N_CORES      # rows per core = 1024
RB = 2                      # r-blocks of 512

CHUNKS = [(126 * k, 126) for k in range(16)] + [(2016, 32)]
NC = len(CHUNKS)            # 17
NS = 2 * NC                 # 34 summary rows (A, C per chunk)

# packed-constant free-dim offsets within cstA [128, CA_W] (bf16)
MOFF = []
_off = 0
for _st, _w in CHUNKS:
    MOFF.append(_off)
    _off += _w
SOFF = _off                 # 2048: ssum_k at SOFF + NS*k, width NS
TOFF = SOFF + NS * NC       # tmat at TOFF, width NS
CA_W = TOFF + NS

_PROGRAM = None


def _build_program():
    nc = bacc.Bacc("TRN2", target_bir_lowering=False, debug=False,
                   num_devices=N_CORES)

    xT_d = nc.declare_dram_parameter("xT", [S, R], BF16, isOutput=False)
    cA_d = nc.declare_dram_parameter("cA", [128, CA_W], BF16, isOutput=False)
    biasT_d = nc.declare_dram_parameter("biasT", [128, NC], F32,
                                        isOutput=False)
    outT_d = nc.declare_dram_parameter("outT", [S, R], BF16, isOutput=True)

    ACT = mybir.ActivationFunctionType

    with tile.TileContext(nc) as tc:
        with (
            tc.tile_pool(name="xp", bufs=1) as xp,
            tc.tile_pool(name="cst", bufs=1) as cst,
            tc.tile_pool(name="sb", bufs=1) as sbp,
            tc.tile_pool(name="osb", bufs=NC) as osb,
            tc.tile_pool(name="pe", bufs=2, space="PSUM") as pep,
            tc.tile_pool(name="po", bufs=6, space="PSUM") as pop,
        ):
            cstA = cst.tile([128, CA_W], BF16, tag="cA")
            nc.gpsimd.dma_start(cstA[:], cA_d[:])
            bias_sb = cst.tile([128, NC], F32, tag="bias")
            nc.gpsimd.dma_start(bias_sb[:], biasT_d[:])

            def ring(k):
                return nc.sync if k % 2 == 0 else nc.scalar

            # ---- x loads: rb-major on the two fast rings ----
            xs = [xp.tile([w + 2, R], BF16, tag=f"x{k}", name=f"x{k}")
                  for k, (st, w) in enumerate(CHUNKS)]
            for rb in range(RB):
                for k, (st, w) in enumerate(CHUNKS):
                    ring(k).dma_start(
                        xs[k][0:w, 512 * rb:512 * (rb + 1)],
                        xT_d[st:st + w, 512 * rb:512 * (rb + 1)])

            E_ps, c_sb = {}, {}

            def emit_summary(k, rb):
                st, w = CHUNKS[k]
                nc.tensor.matmul(
                    E_ps[rb][:],
                    cstA[0:w, SOFF + NS * k:SOFF + NS * (k + 1)],
                    xs[k][0:w, 512 * rb:512 * (rb + 1)],
                    start=(k == 0), stop=(k == NC - 1))

            def emit_transfer(rb):
                # E -> SBUF(bf16) -> transfer matmul -> carries -> SBUF
                e_sb = sbp.tile([NS, 512], BF16, tag=f"esb{rb}",
                                name=f"esb{rb}")
                nc.vector.tensor_copy(e_sb[:], E_ps[rb][:])
                c_ps = pep.tile([NS, 512], F32, tag="pe", name=f"carry{rb}")
                nc.tensor.matmul(c_ps[:], cstA[0:NS, TOFF:TOFF + NS],
                                 e_sb[:], start=True, stop=True)
                t = sbp.tile([NS, 512], BF16, tag=f"csb{rb}",
                             name=f"csb{rb}")
                nc.vector.tensor_copy(t[:], c_ps[:])
                c_sb[rb] = t

            def emit_scatter(rb):
                # carries into the 2 spare partitions of each x tile;
                # gpsimd ring: slow triggers (~0.8us) but keeps the fast
                # SP/Activation rings free for x loads + output stores
                for k, (st, w) in enumerate(CHUNKS):
                    nc.gpsimd.dma_start(
                        xs[k][w:w + 2, 512 * rb:512 * (rb + 1)],
                        c_sb[rb][2 * k:2 * k + 2, :])

            out_sb = {}

            def emit_merged(k, rb):
                st, w = CHUNKS[k]
                ps = pop.tile([w, 512], F32, tag="po", name=f"po{k}_{rb}")
                nc.tensor.matmul(ps[:],
                                 cstA[0:w + 2, MOFF[k]:MOFF[k] + w],
                                 xs[k][0:w + 2, 512 * rb:512 * (rb + 1)],
                                 start=True, stop=True)
                if k not in out_sb:
                    out_sb[k] = osb.tile([w, R], BF16, tag="osb",
                                         name=f"o{k}")
                dst = out_sb[k][:, 512 * rb:512 * (rb + 1)]
                if k % 2 == 0:
                    nc.scalar.activation(dst, ps[:], ACT.Identity,
                                         bias=bias_sb[0:w, k:k + 1])
                else:
                    nc.vector.tensor_scalar_add(dst, ps[:],
                                                bias_sb[0:w, k:k + 1])

            # ---- phase A(rb0): summaries ----
            E_ps[0] = pep.tile([NS, 512], F32, tag="pe", name="E0")
            for k in range(NC):
                emit_summary(k, 0)
            # ---- B(rb0): carries + scatter ----
            emit_transfer(0)
            emit_scatter(0)
            # ---- A(rb1) while rb0 carries settle ----
            E_ps[1] = pep.tile([NS, 512], F32, tag="pe", name="E1")
            for k in range(NC):
                emit_summary(k, 1)
            # ---- C(rb0), with B(rb1) tucked in early ----
            for k in range(NC):
                emit_merged(k, 0)
                if k == 2:
                    emit_transfer(1)
                    emit_scatter(1)
            # ---- C(rb1) + stores ----
            for k in range(NC):
                emit_merged(k, 1)
                st, w = CHUNKS[k]
                ring(k).dma_start(outT_d[st:st + w, :], out_sb[k][:])

    nc.compile()
    return nc


def _host_prep(weight, bias, decay_value):
    w0 = weight[0].astype(np.float64)
    w1 = weight[1].astype(np.float64)
    d0 = float(np.clip(np.float32(decay_value[0, 0]), 0.9, 1.0))
    d1 = float(np.clip(np.float32(decay_value[1, 0]), 0.9, 1.0))

    cA = np.zeros((128, CA_W), dtype=np.float64)
    with np.errstate(under="ignore"):
        for k, (st, w) in enumerate(CHUNKS):
            sig = np.arange(w, dtype=np.float64)[:, None]
            tau = np.arange(w, dtype=np.float64)[None, :]
            m2 = tau >= sig
            p2 = np.where(m2, tau - sig, 0.0) / DC
            w0c = w0[st:st + w]
            w1c = w1[st:st + w]
            mo = MOFF[k]
            cA[0:w, mo:mo + w] = np.where(
                m2, w0c[:, None] * d0 ** p2 + w1c[None, :] * d1 ** p2, 0.0)
            cA[w, mo:mo + w] = d0 ** ((tau[0] + 1) / DC)
            cA[w + 1, mo:mo + w] = w1c * d1 ** ((tau[0] + 1) / DC)
            so = SOFF + NS * k
            cA[0:w, so + 2 * k] = w0c * d0 ** ((w - 1 - sig[:, 0]) / DC)
            cA[0:w, so + 2 * k + 1] = d1 ** ((w - 1 - sig[:, 0]) / DC)
            for kp, (stp, wp) in enumerate(CHUNKS[:k]):
                cA[2 * kp, TOFF + 2 * k] = d0 ** ((st - stp - wp) / DC)
                cA[2 * kp + 1, TOFF + 2 * k + 1] = \
                    d1 ** ((st - stp - wp) / DC)

    biasT = np.zeros((128, NC), dtype=np.float32)
    b32 = bias.astype(np.float32)
    for k, (st, w) in enumerate(CHUNKS):
        biasT[0:w, k] = b32[st:st + w]
    return cA.astype(BF), biasT


def _make_in_maps(x, weight, bias, decay_value):
    cA, biasT = _host_prep(weight, bias, decay_value)
    x2 = np.asarray(x, dtype=np.float32).reshape(B * E, S)
    in_maps = []
    for c in range(N_CORES):
        xT_c = np.ascontiguousarray(x2[R * c:R * (c + 1), :].T.astype(BF))
        in_maps.append({"xT": xT_c, "cA": cA, "biasT": biasT})
    return in_maps


def kernel(x, weight, bias, decay_value, index=0, recurrent=0, **_):
    global _PROGRAM
    x = np.asarray(x, dtype=np.float32)
    weight = np.asarray(weight, dtype=np.float32)
    bias = np.asarray(bias, dtype=np.float32)
    decay_value = np.asarray(decay_value, dtype=np.float32)

    if _PROGRAM is None:
        _PROGRAM = _build_program()
    nc = _PROGRAM

    in_maps = _make_in_maps(x, weight, bias, decay_value)

    res = run_bass_kernel_spmd(nc, in_maps, core_ids=list(range(N_CORES)))
    out = np.empty((B * E, S), dtype=np.float32)
    for c in range(N_CORES):
        out[R * c:R * (c + 1), :] = res.results[c]["outT"].astype(np.float32).T
    return out.reshape(B, E, S)
